# revision 1
# baseline (speedup 1.0000x reference)
"""Single-head causal attention on 8 TRN2 NeuronCores.

Problem shapes (hardcoded): B=8, T=2048, C=1024, H=64, fp32 I/O.
    q = x @ Wq; k = x @ Wk; v = x @ Wv          (per batch element)
    wei = softmax(causal_mask(q @ k.T * C**-0.5))
    out = wei @ v

Sharding: pure data parallel — one batch element per core, no collectives.

Per-core algorithm (matmuls bf16, fp32 PSUM accumulation):
  - host pre-transposes x -> xT [C, T] so C (the contraction dim of the
    QKV projections) lands on SBUF partitions; host packs [Wq|Wk].
  - per 512-wide Tq slice j: qkT = [Wq|Wk].T @ xT (qT rows 0:64, kT rows
    64:128); SBUF->SBUF DMAs build qT2 = [qT;qT] (both partition halves)
    and kT2 (Tk blocks 2m/2m+1 stacked in partition halves) so S^T
    matmuls run ROW-PACKED: two K=64 matmuls execute concurrently in the
    two 64-row halves of the PE array (row_grp packing).
  - vT = Wv.T @ xT; v natural recovered by row-packed identity matmuls,
    stored as v1 = [v | 1] (ones column -> softmax denominators free).
  - S^T pair tiles share one [128,1024] PSUM tensor (2 banks) so the
    exp runs as a single wide ScalarE ACTIVATE where possible.
    P^T = exp(S^T/32); no max subtraction needed (logits std ~0.25;
    softmax is shift invariant). Diagonal 128x128 blocks multiplied by a
    0/1 causal mask; fully-masked column ranges never computed
    (restricted-N matmuls).
  - [out|denom]^T accumulated via lhsT=v1 [128,65], rhs=P^T; epilogue
    transposes 128-col blocks back to natural layout with an identity
    matmul and normalizes per partition (reciprocal + tensor_scalar).
  - projections and attention interleave per j so the PE never idles
    (HAM clock gate stays released); slice j's epilogue is emitted after
    slice j+1's projections so ACT-dependent matmuls never stall the PE
    FIFO head; DMAs are spread across the Sync/GpSimd/Vector queues.
"""

import numpy as np
import ml_dtypes

import concourse.bass as bass
import concourse.mybir as mybir
import concourse.tile as tile
from concourse import bacc
from concourse.bass_utils import run_bass_kernel_spmd

B, T, C, H = 8, 2048, 1024, 64
NCB = C // 128          # 8 C-blocks
NT = T // 128           # 16 Tk-blocks of 128
NJ = T // 512           # 4 Tq-slices of 512
SCALE = float(C) ** -0.5  # 1/32

BF16 = mybir.dt.bfloat16
F32 = mybir.dt.float32
npbf16 = ml_dtypes.bfloat16


def build_attention(nc: bass.Bass, tc: tile.TileContext, ctx):
    xT_d = nc.dram_tensor("xT", [C, T], BF16, kind="ExternalInput").ap()
    wqk_d = nc.dram_tensor("wqk", [C, 128], BF16, kind="ExternalInput").ap()
    wv_d = nc.dram_tensor("wv", [C, H], BF16, kind="ExternalInput").ap()
    out_d = nc.dram_tensor("out", [T, H], F32, kind="ExternalOutput").ap()

    i64_2_np = np.concatenate([np.eye(64, dtype=npbf16)] * 2, axis=0)
    ident64_2 = nc.inline_tensor(i64_2_np, name="ident64_2").ap()
    ident65 = nc.inline_tensor(np.eye(65, dtype=npbf16), name="ident65").ap()
    causal_np = np.triu(np.ones((128, 128), dtype=npbf16))  # keep Tk<=Tq
    causal_d = nc.inline_tensor(causal_np, name="causal").ap()

    consts = ctx.enter_context(tc.tile_pool(name="consts", bufs=1))
    xts = ctx.enter_context(tc.tile_pool(name="xts", bufs=18))
    persist = ctx.enter_context(tc.tile_pool(name="persist", bufs=1))
    pts = ctx.enter_context(tc.tile_pool(name="pts", bufs=6))
    outts = ctx.enter_context(tc.tile_pool(name="outts", bufs=2))
    outs = ctx.enter_context(tc.tile_pool(name="outs", bufs=3))
    smalls = ctx.enter_context(tc.tile_pool(name="smalls", bufs=2))
    ps_qk = ctx.enter_context(tc.tile_pool(name="ps_qk", bufs=1, space="PSUM"))
    ps_vt = ctx.enter_context(tc.tile_pool(name="ps_vt", bufs=1, space="PSUM"))
    ps_big = ctx.enter_context(tc.tile_pool(name="ps_big", bufs=2, space="PSUM"))
    ps_acc = ctx.enter_context(tc.tile_pool(name="ps_acc", bufs=2, space="PSUM"))

    # wqk gates the very first matmul: put it alone on the Scalar queue.
    # Remaining consts go on GpSimd ordered by first use; xt loads own Sync.
    wqk_sb = consts.tile([128, NCB, 128], BF16, tag="wqk")
    nc.scalar.dma_start(out=wqk_sb, in_=wqk_d.rearrange("(c p) h -> p c h", p=128))
    wv_sb = consts.tile([128, NCB, H], BF16, tag="wv")
    nc.gpsimd.dma_start(out=wv_sb, in_=wv_d.rearrange("(c p) h -> p c h", p=128))
    i64_sb = consts.tile([128, 64], BF16, tag="i64")
    nc.gpsimd.dma_start(out=i64_sb, in_=ident64_2)
    causal_sb = consts.tile([128, 128], BF16, tag="causal")
    nc.gpsimd.dma_start(out=causal_sb, in_=causal_d)
    i65_sb = consts.tile([65, 65], BF16, tag="i65")
    nc.gpsimd.dma_start(out=i65_sb, in_=ident65)

    qkT = persist.tile([128, T], BF16, tag="qkT")
    qT2 = persist.tile([128, T], BF16, tag="qT2")       # [qT; qT]
    kT2 = persist.tile([128, T // 2], BF16, tag="kT2")  # Tk pairs in halves
    vT = persist.tile([64, T], BF16, tag="vT")
    vT2 = persist.tile([128, T // 2], BF16, tag="vT2")  # odd Tk blocks, hi half
    v1 = persist.tile([128, NT, H + 1], BF16, tag="v1")  # [v | 1]
    nc.vector.memset(v1, 1.0)

    pending_av = None
    for j in range(NJ):
        jsl = slice(j * 512, (j + 1) * 512)

        # ---- projections for slice j --------------------------------
        xtj = []
        for c in range(NCB):
            xt = xts.tile([128, 512], BF16, tag="xt", name=f"xt{c}_{j}")
            if j > 0:
                eng = nc.sync
            else:  # spread slice-0 loads over three queues for a fast start
                eng = (nc.sync, nc.scalar, nc.gpsimd)[c % 3]
            eng.dma_start(out=xt, in_=xT_d[c * 128:(c + 1) * 128, jsl])
            xtj.append(xt)
        qk_ps = ps_qk.tile([128, 512], F32, tag="qkp", name=f"qk_ps{j}")
        for c in range(NCB):
            nc.tensor.matmul(qk_ps, lhsT=wqk_sb[:, c, :], rhs=xtj[c],
                             start=(c == 0), stop=(c == NCB - 1))
        nc.vector.tensor_copy(qkT[:, jsl], qk_ps)
        # qT into both halves; kT restacked into pair layout (on DVE's
        # queue: FIFO order after the copy above comes for free)
        nc.sync.dma_start(out=qT2[0:64, jsl], in_=qkT[0:64, jsl])
        nc.sync.dma_start(out=qT2[64:128, jsl], in_=qkT[0:64, jsl])
        for b in range(4):  # Tk block 4j+b -> half b%2, col block 2j+b//2
            half = (b % 2) * 64
            c0 = j * 256 + (b // 2) * 128
            nc.gpsimd.dma_start(
                out=kT2[half:half + 64, c0:c0 + 128],
                in_=qkT[64:128, j * 512 + b * 128:j * 512 + (b + 1) * 128])

        vT_ps = ps_vt.tile([64, 512], F32, tag="vtp", name=f"vT_ps{j}")
        for c in range(NCB):
            nc.tensor.matmul(vT_ps, lhsT=wv_sb[:, c, :], rhs=xtj[c],
                             start=(c == 0), stop=(c == NCB - 1))
        nc.vector.tensor_copy(vT[:, jsl], vT_ps)
        for bb in range(2):  # odd Tk blocks 4j+1, 4j+3 -> vT2 hi half
            tb = 4 * j + 2 * bb + 1
            c0 = (2 * j + bb) * 128
            nc.gpsimd.dma_start(
                out=vT2[64:128, c0:c0 + 128],
                in_=vT[:, tb * 128:(tb + 1) * 128])
        # v natural via row-packed identity matmuls (pair of Tk blocks)
        for mt in (2 * j, 2 * j + 1):
            tA, tB = 2 * mt, 2 * mt + 1
            vpA = ps_big.tile([128, H + 1], F32, tag="big", name=f"vpA{mt}")
            vpB = ps_big.tile([128, H + 1], F32, tag="big", name=f"vpB{mt}")
            nc.tensor.matmul(vpA[:, 0:H], lhsT=vT[:, tA * 128:(tA + 1) * 128],
                             rhs=i64_sb[0:64, :], start=True, stop=True)
            nc.tensor.matmul(vpB[:, 0:H],
                             lhsT=vT2[64:128, mt * 128:(mt + 1) * 128],
                             rhs=i64_sb[64:128, :], start=True, stop=True)
            nc.vector.tensor_copy(v1[:, tA, 0:H], vpA[:, 0:H])
            nc.vector.tensor_copy(v1[:, tB, 0:H], vpB[:, 0:H])

        # ---- deferred epilogue of slice j-1 -------------------------
        if pending_av is not None:
            emit_epilogue(nc, outts, outs, smalls, ps_acc, i65_sb, out_d,
                          *pending_av)
            pending_av = None

        # ---- attention for slice j (row-packed S^T, pipelined AV) ---
        av = ps_acc.tile([65, 512], F32, tag="accsm", name=f"av{j}")
        nblk = 4 * j + 4
        prev = None
        for m in range(2 * j + 2):
            sp2 = ps_big.tile([128, 1024], F32, tag="big", name=f"sp{j}_{m}")
            pt2 = pts.tile([128, 1024], BF16, tag="pt", name=f"pt{j}_{m}")
            n0s = []
            for half_idx, i in ((0, 2 * m), (1, 2 * m + 1)):
                g = i - 4 * j
                n0 = max(0, g) * 128
                p0 = half_idx * 64
                o = half_idx * 512
                nc.tensor.matmul(
                    sp2[:, o + n0:o + 512],
                    lhsT=kT2[p0:p0 + 64, m * 128:(m + 1) * 128],
                    rhs=qT2[p0:p0 + 64, j * 512 + n0:(j + 1) * 512],
                    start=True, stop=True)
                n0s.append(n0)
            if n0s[0] == 0 and n0s[1] == 0:  # one wide exp over both banks
                nc.scalar.activation(pt2, sp2,
                                     mybir.ActivationFunctionType.Exp,
                                     scale=SCALE)
            else:
                for half_idx in range(2):
                    o, n0 = half_idx * 512, n0s[half_idx]
                    nc.scalar.activation(
                        pt2[:, o + n0:o + 512], sp2[:, o + n0:o + 512],
                        mybir.ActivationFunctionType.Exp, scale=SCALE)
            for half_idx, i in ((0, 2 * m), (1, 2 * m + 1)):
                g = i - 4 * j
                if g >= 0:  # mask upper triangle of the diagonal block
                    o = half_idx * 512 + n0s[half_idx]
                    nc.vector.tensor_mul(
                        pt2[:, o:o + 128], pt2[:, o:o + 128], causal_sb)
            if prev is not None:
                emit_av(nc, av, v1, *prev, nblk)
            prev = (pt2, n0s, 2 * m)
        emit_av(nc, av, v1, *prev, nblk)
        pending_av = (av, j)

    emit_epilogue(nc, outts, outs, smalls, ps_acc, i65_sb, out_d, *pending_av)


def emit_av(nc, av, v1, pt2, n0s, i0, nblk):
    for d in range(2):
        i = i0 + d
        o, n0 = d * 512, n0s[d]
        nc.tensor.matmul(av[:, n0:512], lhsT=v1[:, i, :],
                         rhs=pt2[:, o + n0:o + 512],
                         start=(i == 0), stop=(i == nblk - 1))


def emit_epilogue(nc, outts, outs, smalls, ps_acc, i65_sb, out_d, av, j):
    osb = outts.tile([65, 512], BF16, tag="osb", name=f"osb{j}")
    nc.vector.tensor_copy(osb, av)  # f32 PSUM -> bf16 SBUF
    for t in range(4):
        op = ps_acc.tile([128, H + 1], F32, tag="accsm", name=f"op{j}_{t}")
        nc.tensor.matmul(op, lhsT=osb[:, t * 128:(t + 1) * 128], rhs=i65_sb,
                         start=True, stop=True)
        rc = smalls.tile([128, 1], F32, tag="rc", name=f"rc{j}_{t}")
        nc.vector.reciprocal(rc, op[:, H:H + 1])
        ot = outs.tile([128, H], F32, tag="ot", name=f"ot{j}_{t}")
        nc.vector.tensor_scalar_mul(ot, op[:, 0:H], rc)
        r0 = (j * 4 + t) * 128
        eng = nc.gpsimd if t % 2 == 0 else nc.sync
        eng.dma_start(out=out_d[r0:r0 + 128, :], in_=ot)


_CACHED = {}


def _get_nc():
    if "nc" not in _CACHED:
        from contextlib import ExitStack
        nc = bacc.Bacc("TRN2", target_bir_lowering=False, debug=False,
                       num_devices=B)
        with tile.TileContext(nc) as tc:
            with ExitStack() as ctx:
                build_attention(nc, tc, ctx)
        nc.compile()
        _CACHED["nc"] = nc
    return _CACHED["nc"]


def kernel(inputs, Wq, Wk, Wv):
    inputs = np.asarray(inputs, dtype=np.float32)
    wqk = np.concatenate([np.asarray(Wq), np.asarray(Wk)], axis=1)
    wqk = np.ascontiguousarray(wqk).astype(npbf16)
    wv = np.ascontiguousarray(np.asarray(Wv)).astype(npbf16)

    in_maps = []
    for b in range(B):
        xT = np.ascontiguousarray(inputs[b].T).astype(npbf16)
        in_maps.append({"xT": xT, "wqk": wqk, "wv": wv})

    nc = _get_nc()
    res = run_bass_kernel_spmd(nc, in_maps, core_ids=list(range(B)))
    out = np.stack([res.results[b]["out"] for b in range(B)], axis=0)
    return out.astype(np.float32)



# revision 10
# speedup vs baseline: 1.0620x; 1.0620x over previous
"""Single-head causal attention on 8 TRN2 NeuronCores.

Problem shapes (hardcoded): B=8, T=2048, C=1024, H=64, fp32 I/O.
    q = x @ Wq; k = x @ Wk; v = x @ Wv          (per batch element)
    wei = softmax(causal_mask(q @ k.T * C**-0.5))
    out = wei @ v
Sharding: pure data parallel -- one batch element per core, no collectives.

Per-core pipeline (matmuls bf16, fp32 PSUM):
  - host packs x as [128, NJ, NCB, 512] (partition, q-slice, C-chunk, t) so
    each 512-wide T-slice loads with ONE fully-contiguous DMA.
  - qkT = [Wq|Wk].T @ xT per slice (8 accumulating MMs, M=128).
  - DVE restacks from PSUM: qT2 = [qT;qT] (tensor_copy + stream_shuffle
    across partition halves), kT2 = k-block pairs stacked in halves.
  - V projection column-packed: even C-chunks -> PE cols 0-63, odd chunks
    -> cols 64-127 concurrently (tile_position), giving vA/vB partial sums;
    v natural recovered per 128-row T-block by row-packed identity MMs
    (rg0: vA chunk, rg1: vB chunk) summed pairwise on DVE into
    v1 = [v | 1] (ones column -> softmax denominators for free).
  - S^T tiles [128, 2, 512] (k-block pair in partition halves): row-packed
    K=64 MMs; ONE strided exp ACTIVATE per pair; causal masks (tri and
    [zeros|tri]) multiplied on GpSimd.
  - AV accumulates [out|denom]^T = v1.T @ P^T with lag-2 emission;
    projection work of slice j+1 is interleaved between attention pairs of
    slice j so the PE never head-of-line blocks on the exp.
  - out written TRANSPOSED + denominator ([65, T] f32); the softmax divide
    and final transpose happen on host (removes PE transposes + epilogue).
  - dummy warm-up matmuls at t=0 release the HAM clock gate early.
"""

import numpy as np
import ml_dtypes

import concourse.bass as bass
import concourse.mybir as mybir
import concourse.tile as tile
from concourse import bacc
from concourse.bass_utils import run_bass_kernel_spmd

B, T, C, H = 8, 2048, 1024, 64
NCB = C // 128          # 8 C-chunks
NT = T // 128           # 16 k-blocks of 128
NJ = T // 512           # 4 q-slices of 512
SCALE = float(C) ** -0.5  # 1/32
N_DUMMY = 16

BF16 = mybir.dt.bfloat16
F32 = mybir.dt.float32
npbf16 = ml_dtypes.bfloat16
IDENT32 = list(range(32))

# cpack layout [128, 448] bf16: i64_2 | tri | zt
CP_I64, CP_TRI, CP_ZT, CP_END = 0, 64, 192, 448


def build_attention(nc: bass.Bass, tc: tile.TileContext, ctx):
    xT_d = nc.dram_tensor("xT", [128, NJ, NCB, 512], BF16,
                          kind="ExternalInput").ap()
    wqk_d = nc.dram_tensor("wqk", [C, 128], BF16, kind="ExternalInput").ap()
    wv_d = nc.dram_tensor("wv", [C, H], BF16, kind="ExternalInput").ap()
    cp_d = nc.dram_tensor("cpack", [128, CP_END], BF16,
                          kind="ExternalInput").ap()
    out_d = nc.dram_tensor("outT", [H + 1, T], F32, kind="ExternalOutput").ap()

    consts = ctx.enter_context(tc.tile_pool(name="consts", bufs=1))
    xpool = ctx.enter_context(tc.tile_pool(name="xpool", bufs=1))
    persist = ctx.enter_context(tc.tile_pool(name="persist", bufs=1))
    qpool = ctx.enter_context(tc.tile_pool(name="qpool", bufs=2))
    vpool = ctx.enter_context(tc.tile_pool(name="vpool", bufs=2))
    pts = ctx.enter_context(tc.tile_pool(name="pts", bufs=4))
    opool = ctx.enter_context(tc.tile_pool(name="opool", bufs=2))
    ps_qk = ctx.enter_context(tc.tile_pool(name="ps_qk", bufs=1, space="PSUM"))
    ps_va = ctx.enter_context(tc.tile_pool(name="ps_va", bufs=1, space="PSUM"))
    ps_vb = ctx.enter_context(tc.tile_pool(name="ps_vb", bufs=1, space="PSUM"))
    ps_s = ctx.enter_context(tc.tile_pool(name="ps_s", bufs=2, space="PSUM"))
    ps_av = ctx.enter_context(tc.tile_pool(name="ps_av", bufs=1, space="PSUM"))

    # ---- t=0: DMAs spread across queues; wqk gates the first real MM ----
    wqk_sb = consts.tile([128, NCB, 128], BF16, tag="wqk")
    nc.scalar.dma_start(out=wqk_sb, in_=wqk_d.rearrange("(c p) h -> p c h",
                                                        p=128))
    dum_sb = consts.tile([128, 128], BF16, tag="dum")
    nc.gpsimd.memset(dum_sb, 0.0)
    wv_sb = consts.tile([128, NCB, H], BF16, tag="wv")
    nc.gpsimd.dma_start(out=wv_sb, in_=wv_d.rearrange("(c p) h -> p c h",
                                                      p=128))
    cp_sb = consts.tile([128, CP_END], BF16, tag="cp")
    nc.gpsimd.dma_start(out=cp_sb, in_=cp_d)
    i64 = cp_sb[:, CP_I64:CP_TRI]     # [128, 64]: I64 stacked twice
    tri = cp_sb[:, CP_TRI:CP_ZT]      # [128, 128] upper-tri (keep k<=q)
    zt = cp_sb[:, CP_ZT:CP_END]       # [128, 256] = [zeros | tri]

    xall = xpool.tile([128, NJ, NCB, 512], BF16, tag="x")
    nc.sync.dma_start(out=xall[:, 0, 0:1, :], in_=xT_d[:, 0, 0:1, :])
    nc.sync.dma_start(out=xall[:, 0, 1:4, :], in_=xT_d[:, 0, 1:4, :])
    nc.sync.dma_start(out=xall[:, 0, 4:8, :], in_=xT_d[:, 0, 4:8, :])
    for j in range(1, NJ):
        nc.sync.dma_start(out=xall[:, j, :, :], in_=xT_d[:, j, :, :])

    kT2 = persist.tile([128, NT // 2, 128], BF16, tag="kT2")
    v1 = persist.tile([128, NT, H + 1], BF16, tag="v1")
    nc.vector.memset(v1, 1.0)

    # ---- HAM warm-up: keep the PE busy while the first DMAs land ----
    dum_ps = ps_va.tile([128, 128], F32, tag="va", name="dum_ps")
    for i in range(N_DUMMY):
        nc.tensor.matmul(dum_ps, lhsT=dum_sb, rhs=dum_sb,
                         start=True, stop=True)

    # ------------------------------------------------------------------
    def emit_prologue(j):
        """Build slice j's projection emission closures (~1-2 PE matmuls
        each, so they can fill exp-wait gaps inside the previous slice's
        attention). Returns (items, qT2)."""
        items = []
        qk = ps_qk.tile([128, 512], F32, tag="qk", name=f"qk{j}")
        for c in range(NCB):
            items.append(lambda c=c, qk=qk: nc.tensor.matmul(
                qk, lhsT=wqk_sb[:, c, :], rhs=xall[:, j, c, :],
                start=(c == 0), stop=(c == NCB - 1)))

        qT2 = qpool.tile([128, 512], BF16, tag="qT2", name=f"qT2_{j}")
        kst = qpool.tile([128, 256], BF16, tag="kst", name=f"kst{j}")

        def restack(qk=qk, qT2=qT2, kst=kst, j=j):
            # stream_shuffle cannot cast (s4d4_tr_same_src_dst_type):
            # cast PSUM->bf16 on matching partitions first, then shuffle.
            nc.vector.tensor_copy(qT2[0:64, :], qk[0:64, :])
            nc.vector.stream_shuffle(qT2[64:128, :], qT2[0:64, :], IDENT32)
            for b in range(4):          # k-block 4j+b -> pair m, half b%2
                m = (4 * j + b) // 2
                src = qk[64:128, b * 128:(b + 1) * 128]
                if b % 2 == 0:          # dest lo half: cast, then shuffle down
                    st = kst[64:128, (b // 2) * 128:(b // 2) * 128 + 128]
                    nc.vector.tensor_copy(st, src)
                    nc.vector.stream_shuffle(kT2[0:64, m, :], st, IDENT32)
                else:                   # dest hi half: direct cast
                    nc.vector.tensor_copy(kT2[64:128, m, :], src)
        items.append(restack)

        # even C-chunks -> PE col group 0-1 -> bank A; odd chunks -> col
        # group 2-3 -> bank B; the two chains run concurrently (separate
        # banks so each keeps its own psum accumulation group)
        vpsa = ps_va.tile([128, 512], F32, tag="va", name=f"vpsa{j}")
        vpsb = ps_vb.tile([128, 512], F32, tag="vb", name=f"vpsb{j}")
        for r in range(4):
            def vproj(r=r, vpsa=vpsa, vpsb=vpsb, j=j):
                nc.tensor.matmul(vpsa[0:64, :], lhsT=wv_sb[:, 2 * r, :],
                                 rhs=xall[:, j, 2 * r, :],
                                 start=(r == 0), stop=(r == 3),
                                 tile_position=(0, 0))
                nc.tensor.matmul(vpsb[64:128, :], lhsT=wv_sb[:, 2 * r + 1, :],
                                 rhs=xall[:, j, 2 * r + 1, :],
                                 start=(r == 0), stop=(r == 3),
                                 tile_position=(0, 64))
            items.append(vproj)

        vth = vpool.tile([128, 512], BF16, tag="vth", name=f"vth{j}")

        def vcast(vpsa=vpsa, vpsb=vpsb, vth=vth):
            nc.vector.tensor_copy(vth[0:64, :], vpsa[0:64, :])
            nc.vector.tensor_copy(vth[64:128, :], vpsb[64:128, :])
        items.append(vcast)

        for t in range(4):
            def vnat(t=t, vth=vth, j=j):
                vpa = ps_va.tile([128, H], F32, tag="va", name=f"vpa{j}_{t}")
                vpb = ps_vb.tile([128, H], F32, tag="vb", name=f"vpb{j}_{t}")
                nc.tensor.matmul(vpa,
                                 lhsT=vth[0:64, t * 128:(t + 1) * 128],
                                 rhs=i64[0:64, :], start=True, stop=True)
                nc.tensor.matmul(vpb,
                                 lhsT=vth[64:128, t * 128:(t + 1) * 128],
                                 rhs=i64[64:128, :], start=True, stop=True)
                # DVE may read only ONE PSUM operand per instruction
                nc.vector.tensor_copy(v1[:, 4 * j + t, 0:H], vpa)
                nc.vector.tensor_add(v1[:, 4 * j + t, 0:H],
                                     v1[:, 4 * j + t, 0:H], vpb)
            items.append(vnat)
        return items, qT2

    # ------------------------------------------------------------------
    def emit_attention(j, qT2, pending):
        """S^T pairs + exp + masks + lag-2 AV, with `pending` (next slice's
        projection items) interleaved between pairs."""
        jsl = slice(j * 512, (j + 1) * 512)
        av = ps_av.tile([H + 1, 512], F32, tag="av", name=f"av{j}")
        npair = 2 * j + 2
        lag = []

        def emit_av(pt, n0, m):
            i_lo, i_hi = 2 * m, 2 * m + 1
            nc.tensor.matmul(av[:, n0:512], lhsT=v1[:, i_lo, :],
                             rhs=pt[:, 0, n0:512],
                             start=(i_lo == 0), stop=False)
            n0h = n0 + 128 if m >= 2 * j else 0
            nc.tensor.matmul(av[:, n0h:512], lhsT=v1[:, i_hi, :],
                             rhs=pt[:, 1, n0h:512],
                             start=False, stop=(i_hi == 4 * j + 3))

        for m in range(npair):
            n0 = (2 * m - 4 * j) * 128 if m >= 2 * j else 0
            sp = ps_s.tile([128, 2, 512], F32, tag="s", name=f"sp{j}_{m}")
            pt = pts.tile([128, 2, 512], BF16, tag="pt", name=f"pt{j}_{m}")
            nc.tensor.matmul(sp[:, 0, n0:512], lhsT=kT2[0:64, m, :],
                             rhs=qT2[0:64, n0:512],
                             start=True, stop=True)
            nc.tensor.matmul(sp[:, 1, n0:512], lhsT=kT2[64:128, m, :],
                             rhs=qT2[64:128, n0:512],
                             start=True, stop=True)
            nc.scalar.activation(pt[:, :, n0:512], sp[:, :, n0:512],
                                 mybir.ActivationFunctionType.Exp,
                                 scale=SCALE)
            if m >= 2 * j:  # diagonal pair: tri on lo half, [0|tri] on hi
                nc.gpsimd.tensor_mul(pt[:, 0, n0:n0 + 128],
                                     pt[:, 0, n0:n0 + 128], tri)
                nc.gpsimd.tensor_mul(pt[:, 1, n0:n0 + 256],
                                     pt[:, 1, n0:n0 + 256], zt)
            lag.append((pt, n0, m))
            # fill PE with next slice's projection work
            if pending:
                take = max(1, -(-len(pending) // (npair - m)))
                for _ in range(min(take, len(pending))):
                    pending.pop(0)()
            if len(lag) > 2:
                emit_av(*lag.pop(0))
        for a in lag:
            emit_av(*a)
        for fn in pending:
            fn()

        outsb = opool.tile([H + 1, 512], F32, tag="osb", name=f"osb{j}")
        nc.vector.tensor_copy(outsb, av)
        nc.sync.dma_start(out=out_d[:, jsl], in_=outsb)

    # ------------------------------------------------------------------
    items0, qT2_0 = emit_prologue(0)
    for fn in items0:   # slice 0 prologue runs inline (nothing to overlap)
        fn()
    qT2s = {0: qT2_0}
    for j in range(NJ):
        if j + 1 < NJ:
            nxt, qT2s[j + 1] = emit_prologue(j + 1)
        else:
            nxt = []
        emit_attention(j, qT2s[j], nxt)


_CACHED = {}


def _get_nc():
    if "nc" not in _CACHED:
        from contextlib import ExitStack
        nc = bacc.Bacc("TRN2", target_bir_lowering=False, debug=False,
                       num_devices=B)
        with tile.TileContext(nc) as tc:
            with ExitStack() as ctx:
                build_attention(nc, tc, ctx)
        nc.compile()
        _CACHED["nc"] = nc
    return _CACHED["nc"]


def make_cpack():
    i64_2 = np.concatenate([np.eye(64, dtype=np.float32)] * 2, axis=0)
    tri = np.triu(np.ones((128, 128), dtype=np.float32))
    zt = np.concatenate([np.zeros((128, 128), np.float32), tri], axis=1)
    cp = np.concatenate([i64_2, tri, zt], axis=1)
    assert cp.shape == (128, CP_END)
    return cp.astype(npbf16)


def prep_in_maps(inputs, Wq, Wk, Wv):
    wqk = np.ascontiguousarray(
        np.concatenate([np.asarray(Wq), np.asarray(Wk)], axis=1)
    ).astype(npbf16)
    wv = np.ascontiguousarray(np.asarray(Wv)).astype(npbf16)
    cp = make_cpack()
    in_maps = []
    for b in range(B):
        xb = np.asarray(inputs[b], dtype=np.float32).astype(npbf16)
        # [T, C] -> [C, T] -> (c p) (j t) -> [p j c t]
        xh = np.ascontiguousarray(
            xb.T.reshape(NCB, 128, NJ, 512).transpose(1, 2, 0, 3))
        in_maps.append({"xT": xh, "wqk": wqk, "wv": wv, "cpack": cp})
    return in_maps


def finish(res):
    outs = []
    for b in range(B):
        oT = np.asarray(res.results[b]["outT"], dtype=np.float32)
        outs.append((oT[:H] / oT[H:H + 1]).T)
    return np.stack(outs, axis=0).astype(np.float32)


def kernel(inputs, Wq, Wk, Wv):
    in_maps = prep_in_maps(np.asarray(inputs), Wq, Wk, Wv)
    nc = _get_nc()
    res = run_bass_kernel_spmd(nc, in_maps, core_ids=list(range(B)))
    return finish(res)
